# revision 11
# baseline (speedup 1.0000x reference)
"""Trainium2 Bass kernel for nn_AIM_8985071583610 (topk_masking).

Math (with the null slot's exact zeros folded away):
  key[b,:]    = x[b] @ key_w.T                  (key_b cancels in the 2-slot softmax)
  query[u,:]  = hs[u] @ query_w[u]
  score[b,u]  = (query[u] . key[b]) / 8
  att0[b,u]   = sigmoid(score[b,u])             (softmax over [s, 0] slots)
  mask[b,u]   = 1 if att0[b,u] in top-32 of row b else 0
  out[b,:]    = sum_u mask[b,u]*att0[b,u] * (x[b] @ hs_value_w[u])

Sharding: units (512) split 64 per core across 8 cores; x/hs replicated as
needed. One in-kernel AllGather of per-core sigmoid blocks gives every core
the global top-32 threshold per row. Host sums the 8 partial outputs.
"""
import sys

if "/opt/trn_rl_repo" not in sys.path:
    sys.path.insert(0, "/opt/trn_rl_repo")

import numpy as np
import ml_dtypes

from concourse import bacc, mybir, tile, masks
from concourse import bass_utils

N_CORES = 8
B = 128
INPUT_SIZE = 256
HIDDEN_SIZE = 512
NUM_UNITS = 512
TOPK = 32
KEY_SIZE = 64
QUERY_SIZE = 64
VALUE_SIZE = 400
U_LOC = NUM_UNITS // N_CORES  # 64 units per core

F32 = mybir.dt.float32
BF16 = mybir.dt.bfloat16

# set True (e.g. from test.py) to capture an NTFF profile; LAST_EXEC_NS is
# filled with neuron-profile exec_time_ns when tracing succeeds.
TRACE = False
LAST_EXEC_NS = None

_CACHED_NC = None


def _build_nc():
    nc = bacc.Bacc("TRN2", target_bir_lowering=False, debug=False, num_devices=N_CORES)

    xt = nc.dram_tensor("xt", [INPUT_SIZE, B], F32, kind="ExternalInput")
    kwt = nc.dram_tensor("kwt", [INPUT_SIZE, KEY_SIZE], F32, kind="ExternalInput")
    hst = nc.dram_tensor("hst", [HIDDEN_SIZE, U_LOC], F32, kind="ExternalInput")
    # query_w shard repacked: [4 h-chunks, 128 h, (64 u x 64 q)]
    qwr = nc.dram_tensor("qwr", [4, 128, U_LOC * QUERY_SIZE], F32, kind="ExternalInput")
    qmask = nc.dram_tensor("qmask", [8, 8 * QUERY_SIZE], F32, kind="ExternalInput")
    w = nc.dram_tensor("w", [U_LOC, INPUT_SIZE, VALUE_SIZE], BF16, kind="ExternalInput")
    out = nc.dram_tensor("out", [B, VALUE_SIZE], F32, kind="ExternalOutput")
    out2 = nc.dram_tensor("out2", [B, VALUE_SIZE], F32, kind="ExternalOutput")

    wflat = w[:].rearrange("u i v -> (u i) v")  # [16384, 400]

    with tile.TileContext(nc) as tc:
        with tc.tile_pool(name="sbuf", bufs=1) as sbuf, \
             tc.tile_pool(name="qwpool", bufs=4) as qwpool, \
             tc.tile_pool(name="wpool", bufs=10) as wpool, \
             tc.tile_pool(name="spool", bufs=3) as spool, \
             tc.tile_pool(name="psq", bufs=2, space="PSUM") as psq, \
             tc.tile_pool(name="psk", bufs=1, space="PSUM") as psk, \
             tc.tile_pool(name="psy", bufs=4, space="PSUM") as psy, \
             tc.tile_pool(name="dram", bufs=1, space="DRAM") as dram:

            # ---- dummy collective fired immediately: absorbs the one-time
            # first-collective init latency concurrently with the load phase.
            dum_in = dram.tile([1, 64], F32)
            dum_out = dram.tile([N_CORES, 64], F32, addr_space="Shared")
            nc.sync.dma_start(dum_in[:], xt[0:1, 0:64])
            nc.gpsimd.collective_compute(
                "AllGather", mybir.AluOpType.bypass,
                replica_groups=[list(range(N_CORES))],
                ins=[dum_in[:].opt()], outs=[dum_out[:].opt()])

            # ---- small input loads
            xts = sbuf.tile([128, 2 * B], F32)      # [p, (c b)]: xt[c*128+p, b]
            nc.sync.dma_start(
                xts[:].rearrange("p (c b) -> p c b", c=2),
                xt[:].rearrange("(c p) b -> p c b", p=128))
            kwts = sbuf.tile([128, 2 * KEY_SIZE], F32)  # [p, (c q)]
            nc.sync.dma_start(
                kwts[:].rearrange("p (c q) -> p c q", c=2),
                kwt[:].rearrange("(c p) q -> p c q", p=128))
            hsts = sbuf.tile([128, 4 * U_LOC], F32)  # [p, (c u)]: hst[c*128+p, u]
            nc.sync.dma_start(
                hsts[:].rearrange("p (c u) -> p c u", c=4),
                hst[:].rearrange("(c p) u -> p c u", p=128))

            acc = sbuf.tile([B, VALUE_SIZE], F32)
            nc.vector.memset(acc[:], 0.0)
            xtb = sbuf.tile([128, 2 * B], BF16)
            nc.vector.tensor_copy(xtb[:], xts[:])

            # ---- qw chunk DMAs (issued before W so they drain first)
            qw_tiles = []
            for c in range(4):
                qt = qwpool.tile([128, U_LOC * QUERY_SIZE], F32, name="qw")
                nc.sync.dma_start(qt[:], qwr[c])
                qw_tiles.append(qt)

            # ---- query einsum, moving-qw form: per 8-unit block j,
            # G[m,(u,q)] = sum_h hs[8j+m,h]*qw[u,h,q]; the diagonal m==u
            # rows are the queries. qw streams as the moving operand
            # (1 pass through PE), hs blocks are the stationary.
            ident = sbuf.tile([128, 128], F32)
            masks.make_identity(nc, ident[:])
            qmk = sbuf.tile([8, 8 * QUERY_SIZE], F32)
            nc.sync.dma_start(qmk[:], qmask[:])
            qrows = sbuf.tile([8, 8 * QUERY_SIZE], F32)
            gm = sbuf.tile([8, 8 * QUERY_SIZE], F32)
            for j in range(8):
                psG = psq.tile([8, 8 * QUERY_SIZE], F32, name="psG")
                for c in range(4):
                    nc.tensor.matmul(
                        psG[:],
                        hsts[:, c * U_LOC + 8 * j: c * U_LOC + 8 * j + 8],
                        qw_tiles[c][:, j * 512:(j + 1) * 512],
                        start=(c == 0), stop=(c == 3))
                # mask out off-diagonal unit blocks, then reduce over u:
                # qrows[m, j*64+q] = sum_u (G[m,(u,q)] * [u==m])
                nc.vector.tensor_tensor(out=gm[:], in0=psG[:], in1=qmk[:],
                                        op=mybir.AluOpType.mult)
                nc.vector.tensor_reduce(
                    out=qrows[:, j * 64:(j + 1) * 64],
                    in_=gm[:].rearrange("p (u q) -> p q u", u=8),
                    axis=mybir.AxisListType.X,
                    op=mybir.AluOpType.add)
            # transpose qrows blocks into queryT [q, u]
            psT = psk.tile([QUERY_SIZE, U_LOC], F32, name="psT", tag="ps_sc")
            for j in range(8):
                nc.tensor.matmul(
                    psT[:, 8 * j:8 * j + 8],
                    qrows[:, j * 64:(j + 1) * 64],
                    ident[0:8, 0:8],
                    is_transpose=True,
                    start=(j == 0), stop=(j == 7))
            queryT = sbuf.tile([QUERY_SIZE, U_LOC], F32)
            nc.vector.tensor_copy(queryT[:], psT[:])

            # ---- keyT = kw @ x (contract input dim), then score + sigmoid
            keyT_ps = psk.tile([KEY_SIZE, B], F32)
            for c in range(2):
                nc.tensor.matmul(keyT_ps[:],
                                 kwts[:, c * KEY_SIZE:(c + 1) * KEY_SIZE],
                                 xts[:, c * B:(c + 1) * B],
                                 start=(c == 0), stop=(c == 1))
            keyT = sbuf.tile([KEY_SIZE, B], F32)
            nc.scalar.copy(keyT[:], keyT_ps[:])

            score_ps = psk.tile([B, U_LOC], F32, tag="ps_sc")
            nc.tensor.matmul(score_ps[:], keyT[:], queryT[:], start=True, stop=True)
            sig_loc = sbuf.tile([B, U_LOC], F32)
            nc.scalar.activation(sig_loc[:], score_ps[:],
                                 mybir.ActivationFunctionType.Sigmoid,
                                 bias=0.0, scale=0.125)

            # ---- local top-32 values (global top-32 is within the union)
            loc32 = sbuf.tile([B, TOPK], F32)
            zap_l = sbuf.tile([B, U_LOC], F32)
            cur = sig_loc
            for it in range(TOPK // 8):
                nc.vector.max(loc32[:, it * 8:(it + 1) * 8], cur[:])
                if it < TOPK // 8 - 1:
                    nc.vector.match_replace(zap_l[:], loc32[:, it * 8:(it + 1) * 8],
                                            cur[:], 0.0)
                    cur = zap_l

            # ---- AllGather local top-32 blocks -> [8*128, 32]
            cc_in = dram.tile([B, TOPK], F32)
            cc_out = dram.tile([N_CORES * B, TOPK], F32, addr_space="Shared")
            nc.scalar.dma_start(cc_in[:], loc32[:])
            nc.gpsimd.collective_compute(
                "AllGather", mybir.AluOpType.bypass,
                replica_groups=[list(range(N_CORES))],
                ins=[cc_in[:].opt()], outs=[cc_out[:].opt()])
            uni = sbuf.tile([B, N_CORES * TOPK], F32)
            for r in range(N_CORES):
                nc.sync.dma_start(uni[:, r * TOPK:(r + 1) * TOPK],
                                  cc_out[r * B:(r + 1) * B, :])

            # ---- global top-32 threshold from the union
            maxb = sbuf.tile([B, 8], F32)
            zap = sbuf.tile([B, N_CORES * TOPK], F32)
            cur = uni
            for it in range(TOPK // 8):
                nc.vector.max(maxb[:], cur[:])
                if it < TOPK // 8 - 1:
                    nc.vector.match_replace(zap[:], maxb[:], cur[:], 0.0)
                    cur = zap
            thr = maxb[:, 7:8]  # 32nd largest sigmoid value, per row

            # gates for my units: (sig >= thr) * sig
            gates = sbuf.tile([B, U_LOC], F32)
            nc.vector.scalar_tensor_tensor(
                out=gates[:], in0=sig_loc[:], scalar=thr, in1=sig_loc[:],
                op0=mybir.AluOpType.is_ge, op1=mybir.AluOpType.mult)

            # ---- value matmuls + gated accumulation. Most units: fused
            # mul-add on DVE straight from PSUM. Every 4th unit: ScalarE
            # scales PSUM->SBUF, GpSimd accumulates, relieving DVE.
            acc_g = sbuf.tile([B, VALUE_SIZE], F32)
            nc.gpsimd.memset(acc_g[:], 0.0)
            for g in range(U_LOC // 4):
                wt = wpool.tile([128, 8 * VALUE_SIZE], BF16, name="wt")
                src = wflat[g * 1024:(g + 1) * 1024].rearrange(
                    "(j p) v -> p j v", p=128)
                nc.sync.dma_start(wt[:].rearrange("p (j v) -> p j v", j=8), src)
                for uu in range(4):
                    u = g * 4 + uu
                    y = psy.tile([B, VALUE_SIZE], F32, name="y")
                    for c in range(2):
                        nc.tensor.matmul(
                            y[:],
                            xtb[:, c * B:(c + 1) * B],
                            wt[:, (uu * 2 + c) * VALUE_SIZE:(uu * 2 + c + 1) * VALUE_SIZE],
                            start=(c == 0), stop=(c == 1))
                    if 4 <= u < 52 and u % 2 == 0:
                        sc = spool.tile([B, VALUE_SIZE], F32, name="sc")
                        nc.scalar.mul(sc[:], y[:], gates[:, u:u + 1])
                        nc.gpsimd.tensor_tensor(out=acc_g[:], in0=sc[:],
                                                in1=acc_g[:],
                                                op=mybir.AluOpType.add)
                    else:
                        nc.vector.scalar_tensor_tensor(
                            out=acc[:], in0=y[:], scalar=gates[:, u:u + 1],
                            in1=acc[:],
                            op0=mybir.AluOpType.mult, op1=mybir.AluOpType.add)
            nc.scalar.dma_start(out[:], acc[:])
            nc.scalar.dma_start(out2[:], acc_g[:])

    nc.compile()
    return nc


def kernel(x, hs, key_w, key_b, hs_value_w, query_w):
    global _CACHED_NC, LAST_EXEC_NS
    x = np.asarray(x, dtype=np.float32)
    hs = np.asarray(hs, dtype=np.float32)
    key_w = np.asarray(key_w, dtype=np.float32)
    hs_value_w = np.asarray(hs_value_w, dtype=np.float32)
    query_w = np.asarray(query_w, dtype=np.float32)

    xt = np.ascontiguousarray(x[:, 0, :].T)          # [256, 128]
    qmask_np = np.zeros((8, 8 * QUERY_SIZE), np.float32)
    for m in range(8):
        qmask_np[m, m * QUERY_SIZE:(m + 1) * QUERY_SIZE] = 1.0
    kwt = np.ascontiguousarray(key_w.T)              # [256, 64]

    in_maps = []
    for c in range(N_CORES):
        U = slice(c * U_LOC, (c + 1) * U_LOC)
        hst = np.ascontiguousarray(hs[U].T)          # [512, 64]
        qwr = np.ascontiguousarray(
            query_w[U].transpose(1, 0, 2)).reshape(4, 128, U_LOC * QUERY_SIZE)
        wshard = np.ascontiguousarray(hs_value_w[U]).astype(ml_dtypes.bfloat16)
        in_maps.append({"xt": xt, "kwt": kwt, "hst": hst, "qwr": qwr, "w": wshard,
                        "qmask": qmask_np})

    if _CACHED_NC is None:
        _CACHED_NC = _build_nc()
    nc = _CACHED_NC

    trace = TRACE
    if trace:
        try:
            from antenv.axon_hooks import set_axon_ntff_profile_hook, \
                get_axon_ntff_profile_hook
            from trn_agent_boot.trn_boot import _ntff_profile_via_ctypes
            if get_axon_ntff_profile_hook() is None:
                set_axon_ntff_profile_hook(
                    _ntff_profile_via_ctypes('/opt/axon/libaxon_pjrt.so'))
        except Exception:
            trace = False

    res = bass_utils.run_bass_kernel_spmd(
        nc, in_maps, core_ids=list(range(N_CORES)), trace=trace)
    LAST_EXEC_NS = res.exec_time_ns

    parts = np.stack([res.results[i]["out"] for i in range(N_CORES)]
                     + [res.results[i]["out2"] for i in range(N_CORES)])
    return np.sum(parts, axis=0, dtype=np.float64).astype(np.float32)


# revision 15
# speedup vs baseline: 1.1138x; 1.1138x over previous
"""Trainium2 Bass kernel for nn_AIM_8985071583610 (topk_masking).

Math (with the null slot's exact zeros folded away):
  key[b,:]    = x[b] @ key_w.T                  (key_b cancels in the 2-slot softmax)
  query[u,:]  = hs[u] @ query_w[u]
  score[b,u]  = (query[u] . key[b]) / 8
  att0[b,u]   = sigmoid(score[b,u])             (softmax over [s, 0] slots)
  mask[b,u]   = 1 if att0[b,u] in top-32 of row b else 0
  out[b,:]    = sum_u mask[b,u]*att0[b,u] * (x[b] @ hs_value_w[u])

Sharding: units (512) split 64 per core across 8 cores; x/hs replicated as
needed. One in-kernel AllGather of per-core sigmoid blocks gives every core
the global top-32 threshold per row. Host sums the 8 partial outputs.
"""
import sys

if "/opt/trn_rl_repo" not in sys.path:
    sys.path.insert(0, "/opt/trn_rl_repo")

import numpy as np
import ml_dtypes

from concourse import bacc, mybir, tile, masks
from concourse import bass_utils

N_CORES = 8
B = 128
INPUT_SIZE = 256
HIDDEN_SIZE = 512
NUM_UNITS = 512
TOPK = 32
KEY_SIZE = 64
QUERY_SIZE = 64
VALUE_SIZE = 400
U_LOC = NUM_UNITS // N_CORES  # 64 units per core

F32 = mybir.dt.float32
BF16 = mybir.dt.bfloat16

# set True (e.g. from test.py) to capture an NTFF profile; LAST_EXEC_NS is
# filled with neuron-profile exec_time_ns when tracing succeeds.
TRACE = False
LAST_EXEC_NS = None

_CACHED_NC = None


def _build_nc():
    nc = bacc.Bacc("TRN2", target_bir_lowering=False, debug=False, num_devices=N_CORES)

    xt = nc.dram_tensor("xt", [INPUT_SIZE, B], F32, kind="ExternalInput")
    kwt = nc.dram_tensor("kwt", [INPUT_SIZE, KEY_SIZE], F32, kind="ExternalInput")
    hst = nc.dram_tensor("hst", [HIDDEN_SIZE, U_LOC], F32, kind="ExternalInput")
    # query_w shard repacked: [4 h-chunks, 128 h, (64 u x 64 q)]
    qwr = nc.dram_tensor("qwr", [4, 128, U_LOC * QUERY_SIZE], F32, kind="ExternalInput")
    qmask = nc.dram_tensor("qmask", [8, 8 * QUERY_SIZE], F32, kind="ExternalInput")
    w = nc.dram_tensor("w", [U_LOC, INPUT_SIZE, VALUE_SIZE], BF16, kind="ExternalInput")
    out = nc.dram_tensor("out", [B, VALUE_SIZE], F32, kind="ExternalOutput")
    out2 = nc.dram_tensor("out2", [B, VALUE_SIZE], F32, kind="ExternalOutput")

    wflat = w[:].rearrange("u i v -> (u i) v")  # [16384, 400]

    with tile.TileContext(nc) as tc:
        with tc.tile_pool(name="sbuf", bufs=1) as sbuf, \
             tc.tile_pool(name="qwpool", bufs=4) as qwpool, \
             tc.tile_pool(name="wpool", bufs=10) as wpool, \
             tc.tile_pool(name="spool", bufs=3) as spool, \
             tc.tile_pool(name="psbig", bufs=6, space="PSUM") as psbig, \
             tc.tile_pool(name="psk", bufs=1, space="PSUM") as psk, \
             tc.tile_pool(name="dram", bufs=1, space="DRAM") as dram:

            # ---- dummy collective fired immediately: absorbs the one-time
            # first-collective init latency concurrently with the load phase.
            dum_in = dram.tile([1, 64], F32)
            dum_out = dram.tile([N_CORES, 64], F32, addr_space="Shared")
            nc.sync.dma_start(dum_in[:], xt[0:1, 0:64])
            nc.gpsimd.collective_compute(
                "AllGather", mybir.AluOpType.bypass,
                replica_groups=[list(range(N_CORES))],
                ins=[dum_in[:].opt()], outs=[dum_out[:].opt()])

            # ---- small input loads
            xts = sbuf.tile([128, 2 * B], F32)      # [p, (c b)]: xt[c*128+p, b]
            nc.sync.dma_start(
                xts[:].rearrange("p (c b) -> p c b", c=2),
                xt[:].rearrange("(c p) b -> p c b", p=128))
            kwts = sbuf.tile([128, 2 * KEY_SIZE], F32)  # [p, (c q)]
            nc.sync.dma_start(
                kwts[:].rearrange("p (c q) -> p c q", c=2),
                kwt[:].rearrange("(c p) q -> p c q", p=128))
            hsts = sbuf.tile([128, 4 * U_LOC], F32)  # [p, (c u)]: hst[c*128+p, u]
            nc.sync.dma_start(
                hsts[:].rearrange("p (c u) -> p c u", c=4),
                hst[:].rearrange("(c p) u -> p c u", p=128))

            acc = sbuf.tile([B, VALUE_SIZE], F32)
            nc.vector.memset(acc[:], 0.0)
            xtb = sbuf.tile([128, 2 * B], BF16)
            nc.vector.tensor_copy(xtb[:], xts[:])

            # ---- qw chunk DMAs (issued before W so they drain first)
            qw_tiles = []
            for c in range(4):
                qt = qwpool.tile([128, U_LOC * QUERY_SIZE], F32, name="qw")
                nc.sync.dma_start(qt[:], qwr[c])
                qw_tiles.append(qt)

            # ---- query einsum, moving-qw form: per 8-unit block j,
            # G[m,(u,q)] = sum_h hs[8j+m,h]*qw[u,h,q]; the diagonal m==u
            # rows are the queries. qw streams as the moving operand
            # (1 pass through PE), hs blocks are the stationary.
            ident = sbuf.tile([128, 128], F32)
            masks.make_identity(nc, ident[:])
            qmk = sbuf.tile([8, 8 * QUERY_SIZE], F32)
            nc.sync.dma_start(qmk[:], qmask[:])
            qrows = sbuf.tile([8, 8 * QUERY_SIZE], F32)
            gm = sbuf.tile([8, 8 * QUERY_SIZE], F32)
            for j in range(8):
                psG = psbig.tile([8, 8 * QUERY_SIZE], F32, name="psG", tag="ps")
                for c in range(4):
                    nc.tensor.matmul(
                        psG[:],
                        hsts[:, c * U_LOC + 8 * j: c * U_LOC + 8 * j + 8],
                        qw_tiles[c][:, j * 512:(j + 1) * 512],
                        start=(c == 0), stop=(c == 3))
                # mask out off-diagonal unit blocks, then reduce over u:
                # qrows[m, j*64+q] = sum_u (G[m,(u,q)] * [u==m])
                nc.vector.tensor_tensor(out=gm[:], in0=psG[:], in1=qmk[:],
                                        op=mybir.AluOpType.mult)
                nc.vector.tensor_reduce(
                    out=qrows[:, j * 64:(j + 1) * 64],
                    in_=gm[:].rearrange("p (u q) -> p q u", u=8),
                    axis=mybir.AxisListType.X,
                    op=mybir.AluOpType.add)
            # transpose qrows blocks into queryT [q, u]
            psT = psk.tile([QUERY_SIZE, U_LOC], F32, name="psT", tag="ps_sc")
            for j in range(8):
                nc.tensor.matmul(
                    psT[:, 8 * j:8 * j + 8],
                    qrows[:, j * 64:(j + 1) * 64],
                    ident[0:8, 0:8],
                    is_transpose=True,
                    start=(j == 0), stop=(j == 7))
            queryT = sbuf.tile([QUERY_SIZE, U_LOC], F32)
            nc.vector.tensor_copy(queryT[:], psT[:])

            # ---- keyT = kw @ x (contract input dim), then score + sigmoid
            keyT_ps = psk.tile([KEY_SIZE, B], F32)
            for c in range(2):
                nc.tensor.matmul(keyT_ps[:],
                                 kwts[:, c * KEY_SIZE:(c + 1) * KEY_SIZE],
                                 xts[:, c * B:(c + 1) * B],
                                 start=(c == 0), stop=(c == 1))
            keyT = sbuf.tile([KEY_SIZE, B], F32)
            nc.scalar.copy(keyT[:], keyT_ps[:])

            score_ps = psk.tile([B, U_LOC], F32, tag="ps_sc")
            nc.tensor.matmul(score_ps[:], keyT[:], queryT[:], start=True, stop=True)
            sig_loc = sbuf.tile([B, U_LOC], F32)
            nc.scalar.activation(sig_loc[:], score_ps[:],
                                 mybir.ActivationFunctionType.Sigmoid,
                                 bias=0.0, scale=0.125)

            # ---- local top-32 values (global top-32 is within the union)
            loc32 = sbuf.tile([B, TOPK], F32)
            zap_l = sbuf.tile([B, U_LOC], F32)
            cur = sig_loc
            for it in range(TOPK // 8):
                nc.vector.max(loc32[:, it * 8:(it + 1) * 8], cur[:])
                if it < TOPK // 8 - 1:
                    nc.vector.match_replace(zap_l[:], loc32[:, it * 8:(it + 1) * 8],
                                            cur[:], 0.0)
                    cur = zap_l

            # ---- AllGather local top-32 blocks -> [8*128, 32]
            cc_in = dram.tile([B, TOPK], F32)
            cc_out = dram.tile([N_CORES * B, TOPK], F32, addr_space="Shared")
            nc.scalar.dma_start(cc_in[:], loc32[:])
            nc.gpsimd.collective_compute(
                "AllGather", mybir.AluOpType.bypass,
                replica_groups=[list(range(N_CORES))],
                ins=[cc_in[:].opt()], outs=[cc_out[:].opt()])
            uni = sbuf.tile([B, N_CORES * TOPK], F32)
            for r in range(N_CORES):
                eng = nc.scalar if r % 2 == 0 else nc.sync
                eng.dma_start(uni[:, r * TOPK:(r + 1) * TOPK],
                              cc_out[r * B:(r + 1) * B, :])

            # ---- global top-32 threshold from the union
            maxb = sbuf.tile([B, 8], F32)
            zap = sbuf.tile([B, N_CORES * TOPK], F32)
            cur = uni
            for it in range(TOPK // 8):
                nc.vector.max(maxb[:], cur[:])
                if it < TOPK // 8 - 1:
                    nc.vector.match_replace(zap[:], maxb[:], cur[:], 0.0)
                    cur = zap
            thr = maxb[:, 7:8]  # 32nd largest sigmoid value, per row

            # gates for my units: (sig >= thr) * sig
            gates = sbuf.tile([B, U_LOC], F32)
            nc.vector.scalar_tensor_tensor(
                out=gates[:], in0=sig_loc[:], scalar=thr, in1=sig_loc[:],
                op0=mybir.AluOpType.is_ge, op1=mybir.AluOpType.mult)

            # ---- value matmuls + gated accumulation. Most units: fused
            # mul-add on DVE straight from PSUM. Every 4th unit: ScalarE
            # scales PSUM->SBUF, GpSimd accumulates, relieving DVE.
            acc_g = sbuf.tile([B, VALUE_SIZE], F32)
            nc.gpsimd.memset(acc_g[:], 0.0)
            for g in range(U_LOC // 4):
                wt = wpool.tile([128, 8 * VALUE_SIZE], BF16, name="wt")
                src = wflat[g * 1024:(g + 1) * 1024].rearrange(
                    "(j p) v -> p j v", p=128)
                nc.sync.dma_start(wt[:].rearrange("p (j v) -> p j v", j=8), src)
                for uu in range(4):
                    u = g * 4 + uu
                    y = psbig.tile([B, VALUE_SIZE], F32, name="y", tag="ps")
                    for c in range(2):
                        nc.tensor.matmul(
                            y[:],
                            xtb[:, c * B:(c + 1) * B],
                            wt[:, (uu * 2 + c) * VALUE_SIZE:(uu * 2 + c + 1) * VALUE_SIZE],
                            start=(c == 0), stop=(c == 1))
                    if 4 <= u < 52 and u % 2 == 0:
                        sc = spool.tile([B, VALUE_SIZE], F32, name="sc")
                        nc.scalar.mul(sc[:], y[:], gates[:, u:u + 1])
                        nc.gpsimd.tensor_tensor(out=acc_g[:], in0=sc[:],
                                                in1=acc_g[:],
                                                op=mybir.AluOpType.add)
                    else:
                        nc.vector.scalar_tensor_tensor(
                            out=acc[:], in0=y[:], scalar=gates[:, u:u + 1],
                            in1=acc[:],
                            op0=mybir.AluOpType.mult, op1=mybir.AluOpType.add)
            nc.scalar.dma_start(out[:], acc[:])
            nc.scalar.dma_start(out2[:], acc_g[:])

    nc.compile()
    return nc


def kernel(x, hs, key_w, key_b, hs_value_w, query_w):
    global _CACHED_NC, LAST_EXEC_NS
    x = np.asarray(x, dtype=np.float32)
    hs = np.asarray(hs, dtype=np.float32)
    key_w = np.asarray(key_w, dtype=np.float32)
    hs_value_w = np.asarray(hs_value_w, dtype=np.float32)
    query_w = np.asarray(query_w, dtype=np.float32)

    xt = np.ascontiguousarray(x[:, 0, :].T)          # [256, 128]
    qmask_np = np.zeros((8, 8 * QUERY_SIZE), np.float32)
    for m in range(8):
        qmask_np[m, m * QUERY_SIZE:(m + 1) * QUERY_SIZE] = 1.0
    kwt = np.ascontiguousarray(key_w.T)              # [256, 64]

    in_maps = []
    for c in range(N_CORES):
        U = slice(c * U_LOC, (c + 1) * U_LOC)
        hst = np.ascontiguousarray(hs[U].T)          # [512, 64]
        qwr = np.ascontiguousarray(
            query_w[U].transpose(1, 0, 2)).reshape(4, 128, U_LOC * QUERY_SIZE)
        wshard = np.ascontiguousarray(hs_value_w[U]).astype(ml_dtypes.bfloat16)
        in_maps.append({"xt": xt, "kwt": kwt, "hst": hst, "qwr": qwr, "w": wshard,
                        "qmask": qmask_np})

    if _CACHED_NC is None:
        _CACHED_NC = _build_nc()
    nc = _CACHED_NC

    trace = TRACE
    if trace:
        try:
            from antenv.axon_hooks import set_axon_ntff_profile_hook, \
                get_axon_ntff_profile_hook
            from trn_agent_boot.trn_boot import _ntff_profile_via_ctypes
            if get_axon_ntff_profile_hook() is None:
                set_axon_ntff_profile_hook(
                    _ntff_profile_via_ctypes('/opt/axon/libaxon_pjrt.so'))
        except Exception:
            trace = False

    res = bass_utils.run_bass_kernel_spmd(
        nc, in_maps, core_ids=list(range(N_CORES)), trace=trace)
    LAST_EXEC_NS = res.exec_time_ns

    parts = np.stack([res.results[i]["out"] for i in range(N_CORES)]
                     + [res.results[i]["out2"] for i in range(N_CORES)])
    return np.sum(parts, axis=0, dtype=np.float64).astype(np.float32)
